# revision 2
# baseline (speedup 1.0000x reference)
"""GroupedQueryAttentionLayer on 8 trn2 NeuronCores (Bass/Tile, SPMD), fp8 edition.

Sharding: data-parallel over query rows; no collectives. Core i handles
batch b = i//4, query rows q0 = (i%4)*512 .. +512. Each core recomputes
its batch's K/V projection; outputs are disjoint row-slices of [2,2048,1024].

Key ideas vs the bf16 baseline:
- All matmul operands in fp8e4 (error budget verified: ~7e-4 vs 2e-2 gate).
- Projections (Q/K/V/post) and PV use perf_mode=DoubleRow: one matmul
  contracts 256 rows (2 k-chunks interleaved), halving PE time there.
- Scores use two concurrent 64-row tiles (tile_position (0,0)/(64,0)):
  K^T for the group is mirrored into both partition halves, the two heads
  of a pair run simultaneously on the upper/lower half of the PE array.
- exp is split across ScalarE (real exp -> fp8) and VectorE (Schraudolph
  int8 bit-trick: round(x*8/ln2 + 56.11) bitcast to fp8e4), since ACT alone
  (1 elem/cycle/lane @1.2GHz) would be the kernel bottleneck.
- V carries a ones column (col 64) so PV emits softmax denominators as
  PSUM row 64; normalize via partition-spread reciprocal + broadcast matmul.
- Post projection accumulates into the residual tile (xres) in SBUF:
  round 0 adds post(m=0,1)+xres, round 1 adds post(m=2,3) and streams out.

Host layouts (k-chunk PAIRS interleaved for DoubleRow, dim order [p,m,u,...]):
  XT   [128,4,2,2048] f8 : XT[p,m,u,t] = X[b,t,(2m+u)*128+p]
  XTQ  [128,4,2, 512] f8 : XT query-column slice
  XRES [128,4,1024] f32  : residual rows (exact fp32)
  WQ/WP [128,4,2,1024] f8, WK/WV [128,4,2,256] f8
Output OUT [4, 128, 1024] f32, OUT[sc, p, :] = row q0 + sc*128 + p.
"""

import math

import numpy as np
import ml_dtypes

F8 = ml_dtypes.float8_e4m3

B, S, D = 2, 2048, 1024
HEADS, GROUPS, E = 16, 4, 64
HPG = HEADS // GROUPS
NCORES = 8
CORES_PER_BATCH = NCORES // B
SLOC = B * S // NCORES
SCALE = 1.0 / math.sqrt(E)

# Schraudolph fp8e4 exp: bits = round(x * 8/ln2 + (7*8 - C)); C tuned so the
# multiplicative sawtooth error is centered (sim-measured).
EXP_A = 8.0 / math.log(2.0)
EXP_B = 56.11

# which t-chunks get DVE (Schraudolph) exp instead of ScalarE
DVE_TCC = frozenset({2, 5, 8, 11, 13, 15})

_prog_cache = {}


def _build_program():
    from contextlib import ExitStack

    import concourse.bacc as bacc
    import concourse.tile as tile
    from concourse import mybir

    f32 = mybir.dt.float32
    b16 = mybir.dt.bfloat16
    f8 = mybir.dt.float8e4
    i8 = mybir.dt.int8
    DR = mybir.MatmulPerfMode.DoubleRow
    Exp = mybir.ActivationFunctionType.Exp

    nc = bacc.Bacc("TRN2", target_bir_lowering=False)

    xt_d = nc.dram_tensor("XT", [128, 4, 2, S], f8, kind="ExternalInput")
    xtq_d = nc.dram_tensor("XTQ", [128, 4, 2, SLOC], f8, kind="ExternalInput")
    xres_d = nc.dram_tensor("XRES", [128, 4, D], f32, kind="ExternalInput")
    wq_d = nc.dram_tensor("WQ", [128, 4, 2, 1024], f8, kind="ExternalInput")
    wk_d = nc.dram_tensor("WK", [128, 4, 2, 256], f8, kind="ExternalInput")
    wv_d = nc.dram_tensor("WV", [128, 4, 2, 256], f8, kind="ExternalInput")
    wp_d = nc.dram_tensor("WP", [128, 4, 2, 1024], f8, kind="ExternalInput")
    out_d = nc.dram_tensor("OUT", [4, 128, D], f32, kind="ExternalOutput")

    with tile.TileContext(nc) as tc, ExitStack() as ctx:
        consts = ctx.enter_context(tc.tile_pool(name="consts", bufs=1))
        work = ctx.enter_context(tc.tile_pool(name="work", bufs=2))
        # PSUM (8 banks): scores 2x2 + pv 2x1 + pp(proj/post/bcast) 2x1
        psA = ctx.enter_context(tc.tile_pool(name="psA", bufs=2, space="PSUM"))
        psS = ctx.enter_context(tc.tile_pool(name="psS", bufs=2, space="PSUM"))
        psV = ctx.enter_context(tc.tile_pool(name="psV", bufs=1, space="PSUM"))

        xt = consts.tile([128, 4, 2, S], f8)
        xtq = consts.tile([128, 4, 2, SLOC], f8)
        xres = consts.tile([128, 4, D], f32)
        wq = consts.tile([128, 4, 2, 1024], f8)
        wk = consts.tile([128, 4, 2, 256], f8)
        wv = consts.tile([128, 4, 2, 256], f8)
        wp = consts.tile([128, 4, 2, 1024], f8)
        kt = consts.tile([128, 4, S], f8)  # K^T per group, mirrored halves
        vpr = consts.tile([128, 8, 2, 4, 80], f8)  # [t, m, u, g, e(+ones at 64)]
        qt = consts.tile([128, 8, SLOC], f8)  # pre-scaled Q^T
        atn = consts.tile([128, 4, 2, SLOC], f8)  # normalized attention A^T
        e64 = consts.tile([128, 128], b16)  # row 64 = 1, else 0 (K=128 bcast)
        rbe = consts.tile([128, 512], b16)  # recip staging, rows != 64 stay 0
        rbo = consts.tile([128, 512], b16)
        warm = consts.tile([128, 8], f32)

        nc.vector.memset(e64[:], 0.0)
        nc.vector.memset(e64[64:65, :], 1.0)
        nc.vector.memset(rbe[:], 0.0)
        nc.vector.memset(rbo[:], 0.0)
        nc.vector.memset(vpr[:], 1.0)
        nc.vector.memset(warm[:], 0.0)
        nc.scalar.activation(warm[:], warm[:], Exp)  # exp table preload

        nc.sync.dma_start(out=wk[:], in_=wk_d[:])
        nc.sync.dma_start(out=xt[:, :, :, 0:512], in_=xt_d[:, :, :, 0:512])
        nc.sync.dma_start(out=xtq[:], in_=xtq_d[:])
        nc.sync.dma_start(out=wq[:, :, :, 0:256], in_=wq_d[:, :, :, 0:256])
        nc.sync.dma_start(out=wv[:], in_=wv_d[:])
        nc.sync.dma_start(out=wq[:, :, :, 256:1024], in_=wq_d[:, :, :, 256:1024])
        for t4 in range(1, 4):
            sl = slice(t4 * 512, (t4 + 1) * 512)
            nc.sync.dma_start(out=xt[:, :, :, sl], in_=xt_d[:, :, :, sl])
        nc.sync.dma_start(out=wp[:], in_=wp_d[:])
        nc.sync.dma_start(out=xres[:], in_=xres_d[:])

        def k_proj(ec):
            """K^T for groups 2ec, 2ec+1 -> kt slots, both partition halves.

            DoubleRow matmuls run at the full 2x rate only when the moving
            operand is <=512 fp8 values wide, so every 512-wide output is
            computed as two 256-wide half chains (same weights, h-inner).
            """
            for tb in range(4):
                ps = psA.tile([128, 512], f32, tag="pp")
                for m in range(4):
                    for h in range(2):
                        hsl = slice(tb * 512 + h * 256, tb * 512 + (h + 1) * 256)
                        nc.tensor.matmul(
                            ps[:, h * 256:(h + 1) * 256],
                            lhsT=wk[:, m, :, ec * 128:(ec + 1) * 128],
                            rhs=xt[:, m, :, hsl],
                            start=(m == 0 and h == 0),
                            stop=(m == 3 and h == 1),
                            perf_mode=DR,
                        )
                sl = slice(tb * 512, (tb + 1) * 512)
                ga, gb = 2 * ec, 2 * ec + 1
                nc.vector.tensor_copy(kt[0:64, ga, sl], ps[0:64, :])
                nc.vector.tensor_copy(kt[64:128, gb, sl], ps[64:128, :])
                nc.gpsimd.dma_start(out=kt[64:128, ga, sl], in_=kt[0:64, ga, sl])
                nc.gpsimd.dma_start(out=kt[0:64, gb, sl], in_=kt[64:128, gb, sl])

        def q_proj(hc):
            ps = psA.tile([128, 512], f32, tag="pp")
            for m in range(4):
                for h in range(2):
                    nc.tensor.matmul(
                        ps[:, h * 256:(h + 1) * 256],
                        lhsT=wq[:, m, :, hc * 128:(hc + 1) * 128],
                        rhs=xtq[:, m, :, h * 256:(h + 1) * 256],
                        start=(m == 0 and h == 0),
                        stop=(m == 3 and h == 1),
                        perf_mode=DR,
                    )
            nc.vector.tensor_scalar_mul(qt[:, hc, :], ps, SCALE)

        def v_proj_tcc(tcc):
            ps = psA.tile([128, 256], f32, tag="pp")
            for m in range(4):
                nc.tensor.matmul(
                    ps[:],
                    lhsT=xt[:, m, :, tcc * 128:(tcc + 1) * 128],
                    rhs=wv[:, m, :, :],
                    start=(m == 0),
                    stop=(m == 3),
                    perf_mode=DR,
                )
            nc.vector.tensor_copy(
                vpr[:, tcc // 2, tcc % 2, :, 0:E],
                ps.rearrange("p (g e) -> p g e", g=4),
            )

        exbs = {}  # (c, m) -> exb tile [128, j, u, 512]

        def scores(c, tcc):
            g = c // 2
            ps2 = psS.tile([128, 2, 512], f32, tag="sc", name="ps2")
            tsl = slice(tcc * 128, (tcc + 1) * 128)
            nc.tensor.matmul(
                ps2[:, 0, :], lhsT=kt[0:64, g, tsl], rhs=qt[0:64, c, :],
                start=True, stop=True, tile_position=(0, 0),
            )
            nc.tensor.matmul(
                ps2[:, 1, :], lhsT=kt[64:128, g, tsl], rhs=qt[64:128, c, :],
                start=True, stop=True, tile_position=(64, 0),
            )
            m, u = tcc // 2, tcc % 2
            if u == 0:
                # layout [p, u, j, q]: exp writes a fully contiguous
                # [128, 1024] slab; PV reads [128, 2, 512] with stride 1024
                exbs[(c, m)] = work.tile(
                    [128, 2, 2, 512], f8, tag="exb", bufs=6, name="exb"
                )
            exb = exbs[(c, m)]
            if tcc in DVE_TCC:
                nc.vector.tensor_scalar(
                    exb.bitcast(i8)[:, u, :, :], ps2[:], EXP_A, EXP_B,
                    mybir.AluOpType.mult, mybir.AluOpType.add,
                )
            else:
                nc.scalar.activation(exb[:, u, :, :], ps2[:], Exp)

        state = {}  # live psV tiles per pair: c -> (pve, pvo)

        def pv_mm(c, m):
            g = c // 2
            if m == 0:
                state[c] = (
                    psV.tile([E + 1, 512], f32, tag="pve", name="pve"),
                    psV.tile([E + 1, 512], f32, tag="pvo", name="pvo"),
                )
            pve, pvo = state[c]
            exb = exbs.pop((c, m)) if m == 7 else exbs[(c, m)]
            for j, pv in ((0, pve), (1, pvo)):
                for h in range(2):
                    nc.tensor.matmul(
                        pv[:, h * 256:(h + 1) * 256],
                        lhsT=vpr[:, m, :, g, 0:E + 1],
                        rhs=exb[:, :, j, h * 256:(h + 1) * 256],
                        start=(m == 0 and h == 0),
                        stop=(m == 7 and h == 1),
                        perf_mode=DR,
                    )

        aun = {}

        def pv_evict(c):
            """A'+den to SBUF right after PV stop frees the psV slots early."""
            pve, pvo = state.pop(c)
            te = work.tile([E + 1, 512], b16, tag="aune", name="aune")
            to = work.tile([E + 1, 512], b16, tag="auno", name="auno")
            nc.vector.tensor_copy(te[:], pve[:])
            nc.vector.tensor_copy(to[:], pvo[:])
            aun[c] = (te, to)

        def recips(c):
            te, to = aun[c]
            for t, rb in ((te, rbe), (to, rbo)):
                # spread the 512 denominators over 64 partitions so the DVE
                # reciprocal runs at 8 elements/lane instead of 512
                dsp = work.tile([64, 8], b16, tag="dsp")
                nc.gpsimd.dma_start(
                    out=dsp[:, None, :],
                    in_=t[E:E + 1, :].rearrange("p (a b) -> p a b", a=64),
                )
                rsp = work.tile([64, 8], b16, tag="rsp")
                with nc.allow_low_precision(reason="bf16 softmax recip"):
                    nc.vector.reciprocal(rsp[:], dsp[:])
                nc.gpsimd.dma_start(
                    out=rb[E:E + 1, :].rearrange("p (a b) -> p a b", a=64),
                    in_=rsp[:, None, :],
                )

        def norm_head(c, j):
            te, to = aun[c]
            t, rb = (te, rbe) if j == 0 else (to, rbo)
            bc = psA.tile([128, 512], f32, tag="pp")
            nc.tensor.matmul(bc[:], lhsT=e64[:], rhs=rb[:], start=True, stop=True)
            m, u = c // 2, c % 2
            if j == 0:
                nc.vector.tensor_mul(atn[0:64, m, u, :], t[0:64, :], bc[0:64, :])
            else:
                so = work.tile([64, 512], f8, tag="so")
                nc.vector.tensor_mul(so[:], t[0:64, :], bc[0:64, :])
                nc.gpsimd.dma_start(out=atn[64:128, m, u, :], in_=so[:])
                aun.pop(c)

        def post_slot(i):
            """Post projection for output slot i (all 4 m-pairs), then add
            the residual and stream the finished rows out."""
            sc, dc = i // 2, i % 2
            dsl = slice(dc * 512, (dc + 1) * 512)
            pp = psA.tile([128, 512], f32, tag="pp")
            for m in range(4):
                for h in range(2):
                    nc.tensor.matmul(
                        pp[:, h * 256:(h + 1) * 256],
                        lhsT=atn[:, m, :, sc * 128:(sc + 1) * 128],
                        rhs=wp[:, m, :, dc * 512 + h * 256:dc * 512 + (h + 1) * 256],
                        start=(m == 0 and h == 0),
                        stop=(m == 3 and h == 1),
                        perf_mode=DR,
                    )
            ores = work.tile([128, 512], f32, tag="or")
            nc.vector.tensor_add(ores[:], pp[:], xres[:, sc, dsl])
            nc.sync.dma_start(out=out_d[sc, :, dsl], in_=ores[:])

        def pair_blocks(c):
            """One pipeline step: scores/exp for pair c, trailing PV for c,
            and the tail (PV finish, evict, normalize, posts) of pair c-1."""
            for tcb in range(8):
                if c < 8:
                    if c == 0:
                        v_proj_tcc(2 * tcb)
                        v_proj_tcc(2 * tcb + 1)
                    scores(c, 2 * tcb)
                    scores(c, 2 * tcb + 1)
                if c > 0:
                    b = c - 1
                    if tcb == 0:
                        pv_mm(b, 6)
                        pv_mm(b, 7)
                        pv_evict(b)
                    elif tcb == 1:
                        recips(b)
                    elif tcb == 2:
                        norm_head(b, 0)
                    elif tcb == 3:
                        norm_head(b, 1)
                    elif c == 8 and tcb >= 4:
                        post_slot(2 * (tcb - 4))
                        post_slot(2 * (tcb - 4) + 1)
                if c < 6 and tcb == 5:
                    q_proj(c + 2)
                if c < 8 and tcb >= 2:
                    pv_mm(c, tcb - 2)

        k_proj(0)
        q_proj(0)
        pair_blocks(0)
        q_proj(1)
        pair_blocks(1)
        k_proj(1)
        for c in range(2, 8):
            pair_blocks(c)
        pair_blocks(8)  # drain: tail of pair 7 + post round 1 + output

    nc.compile()
    return nc


def get_program():
    if "nc" not in _prog_cache:
        _prog_cache["nc"] = _build_program()
    return _prog_cache["nc"]


def _pair_chunk(a):
    """[n_in, n_out] -> [128, 4, 2, n_out] with in-dim split (2m+u)*128+p."""
    n = a.shape[1]
    return np.ascontiguousarray(a.reshape(4, 2, 128, n).transpose(2, 0, 1, 3))


def make_in_maps(X, Wq, Wk, Wv, Wpost):
    X = np.asarray(X, dtype=np.float32)
    wq_p = _pair_chunk(np.asarray(Wq, dtype=np.float32)).astype(F8)
    wk_p = _pair_chunk(np.asarray(Wk, dtype=np.float32)).astype(F8)
    wv_p = _pair_chunk(np.asarray(Wv, dtype=np.float32)).astype(F8)
    wp_p = _pair_chunk(np.asarray(Wpost, dtype=np.float32)).astype(F8)

    xt_b = []
    for b in range(B):
        xt_b.append(_pair_chunk(np.ascontiguousarray(X[b].T)).astype(F8))

    in_maps = []
    for core in range(NCORES):
        b = core // CORES_PER_BATCH
        q0 = (core % CORES_PER_BATCH) * SLOC
        xt = xt_b[b]
        xres = np.ascontiguousarray(
            X[b, q0:q0 + SLOC].reshape(4, 128, D).transpose(1, 0, 2)
        )
        in_maps.append(
            {
                "XT": xt,
                "XTQ": np.ascontiguousarray(xt[:, :, :, q0:q0 + SLOC]),
                "XRES": xres,
                "WQ": wq_p,
                "WK": wk_p,
                "WV": wv_p,
                "WP": wp_p,
            }
        )
    return in_maps


def assemble_output(results):
    out = np.empty((B, S, D), dtype=np.float32)
    for core, r in enumerate(results):
        b = core // CORES_PER_BATCH
        q0 = (core % CORES_PER_BATCH) * SLOC
        out[b, q0:q0 + SLOC] = np.asarray(r["OUT"]).reshape(SLOC, D)
    return out


def kernel(X, Wq, Wk, Wv, Wpost, _trace=False):
    from concourse.bass_utils import run_bass_kernel_spmd

    nc = get_program()
    in_maps = make_in_maps(X, Wq, Wk, Wv, Wpost)
    res = run_bass_kernel_spmd(nc, in_maps, core_ids=list(range(NCORES)), trace=_trace)
    out = assemble_output(res.results)
    if _trace:
        return out, res
    return out
